# revision 19
# baseline (speedup 1.0000x reference)
"""Trainium2 kernel for nn_AttentionConstrainedLoss.

Strategy (8 NeuronCores, full inputs in / full output out):
  - The memory-heavy part is the per-grid unbiased variance over D=128 of
    atten_map [B=4, HW=65536, D=128] fp32 (128 MiB).  Sharding: data-parallel
    over B (4 scenes) x sequence-sharded over HW (2 halves) = 8 cores, each
    streaming a 16 MiB chunk against the ~47us DMA roofline.
  - Reduction: one DVE bn_stats per PAIR of grid cells, with a 3D access
    pattern [128, d=128, c=2] (innermost = cell) that interleaves the two
    cells element-wise in scan order.  bn_stats' hardware semantics produce
    count/mean/M2 separately for even and odd scan positions, i.e. exactly
    per-cell M2 for both cells in one 327ns instruction (164ns/cell - the
    only engine pairing that keeps reduction throughput under the DMA
    roofline with slack).  The raw 6-tuple stats are stored to DRAM and the
    host extracts M2_even/M2_odd (unbiased variance = M2/127).  No combine,
    no ACT/Pool work, no second pass on device.
  - The per-pair instruction is emitted as a raw InstBNStats (the bass.py
    wrapper mis-models multi-dim inputs as per-row stats; walrus requires
    exactly 6 outputs/partition = whole-scan even/odd, which is what the
    hardware computes).
  - Input streams as a taper: [4,4] + [8]*26 + [6] + [4]*7 + [2]*3 pieces.
    Tile gates DMA issue on the 8 HWDGE-lane completion sems (issue of DMA
    #n waits completion of #n-8, +900ns sem + ~1.9us issue path) and the
    issue machinery itself costs ~650ns/DMA, so pieces taper only down to
    2 cells at the very end.  The shrinking pieces keep the endgame
    sem-gated: the last pair's stats land ~1.5us after the final byte
    (900ns DMA-sem + 335ns bn_stats + residual lag).
  - Stores: bulk stats (pairs 0-119) from the idle ACT queue - its request
    lands just after the last input piece's so it never preempts the
    stream; the 8-pair tail from SP right after the last bn_stats.  The
    two stats halves live in separate SBUF tiles and DRAM tensors because
    Tile's overlap tracking is tile-granular (a shared tile would add a
    write-after-read edge from the bulk store onto later bn_stats).
  - The box -> grid assignment (point-in-rotated-rect over a 0.4 m grid,
    sequential overlap-kill scan, segment means) touches only ~400 cells per
    box (boxes are <= 5 m).  It is exact, tiny, and done on host in fp32
    numpy replicating the reference semantics including scan order and
    argmin tie-breaking.
"""

import numpy as np

# ---------------------------------------------------------------------------
# Problem constants (hardcoded per contract; kernel.py must be self-contained)
# ---------------------------------------------------------------------------
B, M, D = 4, 100, 128
H, W = 256, 256
HW = H * W
N_CORES = 8
HALF = HW // 2  # grid rows per core (sequence shard)
P = 128  # SBUF partitions
TPP = HALF // P  # grid cells per partition per core (256)
NPAIR = TPP // 2  # bn_stats instructions per core (2 cells each)

_PC_RANGE = np.asarray([-51.2, -51.2, -5.0, 51.2, 51.2, 3.0], dtype=np.float32)
_DIMS = _PC_RANGE[3:] - _PC_RANGE[:3]
_EFF_MIN, _EFF_MAX = np.float32(1.0), np.float32(6.0)

_NC_CACHE = {}
_CFG = {}


def _piece_plan():
    plan = _CFG.get("pieces")
    if plan is None:
        # min piece = 4 cells: the issue path (650ns SP.SEQ + 625ns HWDGE
        # per DMA) cannot sustain smaller pieces back-to-back
        plan = [4, 4] + [8] * 26 + [6] + [4] * 7 + [2] * 3
    assert sum(plan) == TPP and all(s % 2 == 0 for s in plan)
    return plan


def _build_bass_program():
    """Per-core program: atten chunk [32768, 128] f32 -> stats [128, 768] f32.

    Partition layout: grid cell g_local = p * 256 + t  (p = partition,
    t = free index).  Each partition reads 256*128 = 32768 contiguous fp32
    from HBM, so DMA descriptors stay fully contiguous per partition.
    """
    import concourse.bacc as bacc
    import concourse.mybir as mybir
    from concourse import tile

    f32 = mybir.dt.float32

    nc = bacc.Bacc("TRN2", target_bir_lowering=False, debug=False)
    atten = nc.dram_tensor("atten", [HALF, D], f32, kind="ExternalInput")
    bulk_pairs = int(_CFG.get("bulk_pairs", 120))
    bulk_at = int(_CFG.get("bulk_at", 33))  # emit bulk store after this piece
    tail_pairs = NPAIR - bulk_pairs
    # separate DRAM outs + separate SBUF stats tiles: Tile's overlap tracking
    # is tile-granular, so a shared tile would put a write-after-read edge
    # from the bulk store onto every later bn_stats (a ~10us DVE stall)
    s_out_a = nc.dram_tensor(
        "s_out_a", [P, bulk_pairs * 6], f32, kind="ExternalOutput"
    )
    s_out_b = nc.dram_tensor(
        "s_out_b", [P, tail_pairs * 6], f32, kind="ExternalOutput"
    )

    # [128, 32768] view: partition p <- rows [p*256, (p+1)*256), contiguous.
    av = atten[:, :].rearrange("(p t) d -> p (t d)", p=P)

    pieces = _piece_plan()

    def pair_bn_stats(in_ap, out_ap):
        # raw InstBNStats: whole-scan even/odd stats (6 per partition).
        inst = mybir.InstBNStats(
            name=nc.get_next_instruction_name(),
            ins=[nc.vector.lower_ap(in_ap)],
            outs=[nc.vector.lower_ap(out_ap)],
        )
        return nc.vector.add_instruction(inst)

    with tile.TileContext(nc) as tc:
        with (
            tc.tile_pool(name="io", bufs=int(_CFG.get("bufs", 12))) as io_pool,
            tc.tile_pool(name="acc", bufs=1) as acc_pool,
        ):
            stats_a = acc_pool.tile([P, bulk_pairs * 6], f32, tag="stats_a")
            stats_b = acc_pool.tile([P, tail_pairs * 6], f32, tag="stats_b")

            cell = 0
            for pi, n in enumerate(pieces):
                slab = io_pool.tile([P, n * D], f32, tag=f"slab{n}")
                nc.sync.dma_start(
                    out=slab[:], in_=av[:, cell * D : (cell + n) * D]
                )
                for k in range(0, n, 2):
                    pr = (cell + k) // 2
                    if pr < bulk_pairs:
                        st = stats_a[:, pr * 6 : (pr + 1) * 6]
                    else:
                        st = stats_b[:, (pr - bulk_pairs) * 6 : (pr - bulk_pairs + 1) * 6]
                    # interleaved scan: innermost dim = cell (stride 128),
                    # so even positions = cell 2pr, odd = cell 2pr+1
                    in3 = slab[:, k * D : (k + 2) * D].rearrange(
                        "p (c d) -> p d c", c=2
                    )
                    pair_bn_stats(in3, st)
                cell += n
                if pi == bulk_at:
                    # bulk stats store from the otherwise-idle ACT queue;
                    # waits pair bulk_pairs-1's bn_stats via Tile deps
                    nc.scalar.dma_start(out=s_out_a[:, :], in_=stats_a[:])

            # tail store right after the last bn_stats
            nc.sync.dma_start(out=s_out_b[:, :], in_=stats_b[:])

    nc.compile()
    return nc


def _get_nc():
    if "nc" not in _NC_CACHE:
        _NC_CACHE["nc"] = _build_bass_program()
    return _NC_CACHE["nc"]


def _device_variance(atten_map: np.ndarray, trace: bool = False):
    """Run the SPMD kernel on 8 cores. Returns per-grid M2 [B, HW] f32
    (unbiased variance times 127; scaled on host)."""
    from concourse.bass_utils import run_bass_kernel_spmd

    nc = _get_nc()
    in_maps = []
    for c in range(N_CORES):
        b, h = c // 2, c % 2
        # slice BEFORE materializing so jax-array inputs transfer in 16 MiB
        # per-core pieces (large single device->host copies can fail)
        chunk = atten_map[b, h * HALF : (h + 1) * HALF, :]
        chunk = np.ascontiguousarray(np.asarray(chunk), dtype=np.float32)
        in_maps.append({"atten": chunk})
    res = run_bass_kernel_spmd(nc, in_maps, list(range(N_CORES)), trace=trace)
    v = np.empty((B, HW), dtype=np.float32)
    for c in range(N_CORES):
        b, h = c // 2, c % 2
        sa = res.results[c]["s_out_a"].reshape(P, -1, 6)
        sb = res.results[c]["s_out_b"].reshape(P, -1, 6)
        st = np.concatenate([sa, sb], axis=1)
        vc = np.empty((P, TPP), dtype=np.float32)
        vc[:, 0::2] = st[:, :, 2]  # M2 of even scan positions = cell 2k
        vc[:, 1::2] = st[:, :, 5]  # M2 of odd scan positions = cell 2k+1
        v[b, h * HALF : (h + 1) * HALF] = vc.reshape(HALF)
    return v, res


# ---------------------------------------------------------------------------
# Host-side box logic (exact fp32 replication of the reference semantics)
# ---------------------------------------------------------------------------
def _grid_axis_vals():
    gx = (np.arange(W, dtype=np.float32) + np.float32(0.5)) / np.float32(W) * _DIMS[
        0
    ] + _PC_RANGE[0]
    gy = (np.arange(H, dtype=np.float32) + np.float32(0.5)) / np.float32(H) * _DIMS[
        1
    ] + _PC_RANGE[1]
    return gx, gy


_CORNERS_NORM = np.asarray(
    [[-0.5, -0.5], [-0.5, 0.5], [0.5, 0.5], [0.5, -0.5]], dtype=np.float32
)


def _scene_loss(v: np.ndarray, boxes: np.ndarray, gx: np.ndarray, gy: np.ndarray):
    centers = boxes[:, :2]
    lw = boxes[:, 3:5]
    angles = boxes[:, 6]
    ratio_l = np.clip(_DIMS[0] / np.float32(W) / lw[:, 0], _EFF_MIN, _EFF_MAX)
    ratio_w = np.clip(_DIMS[1] / np.float32(H) / lw[:, 1], _EFF_MIN, _EFF_MAX)
    eff = np.stack([lw[:, 0] * ratio_l, lw[:, 1] * ratio_w], axis=1)
    corners = eff[:, None, :] * _CORNERS_NORM  # [M, 4, 2]
    c = np.cos(angles)[:, None]
    s = np.sin(angles)[:, None]
    rx = corners[..., 0] * c + corners[..., 1] * s
    ry = -corners[..., 0] * s + corners[..., 1] * c
    corners = np.stack([rx, ry], axis=-1) + centers[:, None, :]  # [M, 4, 2]
    edges = np.roll(corners, -1, axis=1) - corners

    # exact argmin (first-index tie-break) of d2 over the full grid, as in ref
    d2 = (gx[None, None, :] - centers[:, 0:1, None]) ** 2 + (
        gy[None, :, None] - centers[:, 1:2, None]
    ) ** 2  # [M, H, W] f32
    nearest_g = np.argmin(d2.reshape(M, HW), axis=1)

    flag = np.full(HW, -1, dtype=np.int32)
    for i in range(M):
        cmin, cmax = corners[i, :, 0].min(), corners[i, :, 0].max()
        rmin, rmax = corners[i, :, 1].min(), corners[i, :, 1].max()
        c0 = max(0, int(np.searchsorted(gx, cmin)) - 1)
        c1 = min(W, int(np.searchsorted(gx, cmax)) + 1)
        r0 = max(0, int(np.searchsorted(gy, rmin)) - 1)
        r1 = min(H, int(np.searchsorted(gy, rmax)) + 1)
        dx = gx[None, None, c0:c1] - corners[i, :, 0][:, None, None]
        dy = gy[None, r0:r1, None] - corners[i, :, 1][:, None, None]
        cross = (
            edges[i, :, 0][:, None, None] * dy - edges[i, :, 1][:, None, None] * dx
        )
        inside = np.all(cross >= 0, axis=0) | np.all(cross <= 0, axis=0)
        rr, cc = np.nonzero(inside)
        gidx = (rr + r0).astype(np.int64) * W + (cc + c0)
        gidx = np.union1d(gidx, np.asarray([nearest_g[i]]))
        cur = flag[gidx]
        flag[gidx] = np.where(cur == -1, np.int32(i), np.int32(-1))

    sums = np.zeros(M, dtype=np.float32)
    cnts = np.zeros(M, dtype=np.float32)
    msk = flag >= 0
    np.add.at(sums, flag[msk], v[msk])
    np.add.at(cnts, flag[msk], np.float32(1.0))
    sums *= np.float32(1.0 / 127.0)  # device emits M2; unbiased var = M2/127
    valid = cnts > 0
    box_mean = sums / np.maximum(cnts, np.float32(1.0))
    loss = -np.sum(box_mean[valid], dtype=np.float32)
    return loss, np.float32(np.sum(valid))


def _host_reduce(v: np.ndarray, gt_bboxes: np.ndarray):
    gx, gy = _grid_axis_vals()
    losses = np.zeros(B, dtype=np.float32)
    nums = np.zeros(B, dtype=np.float32)
    for b in range(B):
        losses[b], nums[b] = _scene_loss(
            v[b], np.asarray(gt_bboxes[b], dtype=np.float32), gx, gy
        )
    var_loss = np.sum(losses, dtype=np.float32)
    var_pos_num = np.maximum(np.sum(nums, dtype=np.float32), np.float32(1.0))
    return np.asarray(np.float32(var_loss / var_pos_num))


def kernel(atten_map: np.ndarray, gt_bboxes: np.ndarray, gt_labels: np.ndarray):
    gt_bboxes = np.asarray(gt_bboxes, dtype=np.float32)
    v, _ = _device_variance(atten_map)
    return _host_reduce(v, gt_bboxes)


# revision 20
# speedup vs baseline: 4.5314x; 4.5314x over previous
"""Trainium2 kernel for nn_AttentionConstrainedLoss.

Strategy (8 NeuronCores, full inputs in / full output out):
  - Key sparsity: the loss only reads the per-grid variance of cells COVERED
    by a box after the overlap-kill scan (flag >= 0) - ~1.2-1.5k of 32k
    cells per core shard for this input regime.  The box -> grid assignment
    is exact, tiny (boxes <= 5 m on a 0.4 m grid) and runs on host FIRST;
    the device then gathers only the covered cells' feature rows.
  - Sharding: data-parallel over B (4 scenes) x sequence-sharded over HW
    (2 halves) = 8 cores.  Each core receives its full 16 MiB shard in DRAM
    plus an int16 index table of covered cells, and dma_gathers just those
    rows (512 B descriptors, full DMA bandwidth) in 2-3 chunks pipelined
    against GPSIMD descriptor generation.
  - Reduction: one DVE bn_stats per PAIR of gathered cells, with a 3D
    access pattern [128, d=128, c=2] (innermost = cell) that interleaves
    the two cells element-wise in scan order; the hardware's even/odd-lane
    stats then give per-cell count/mean/M2 for both cells in one 327ns
    instruction.  Raw 6-tuple stats go to DRAM; the host extracts
    M2_even/M2_odd (unbiased variance = M2/127) and scatters them into the
    (host-exact) segment reduction.  Emitted as raw InstBNStats: walrus
    requires exactly 6 outputs/partition = whole-scan even/odd semantics.
  - The host reduction replicates the reference fp32 semantics exactly,
    including scan order and argmin tie-breaking.
"""

import numpy as np

# ---------------------------------------------------------------------------
# Problem constants (hardcoded per contract; kernel.py must be self-contained)
# ---------------------------------------------------------------------------
B, M, D = 4, 100, 128
H, W = 256, 256
HW = H * W
N_CORES = 8
HALF = HW // 2  # grid rows per core (sequence shard)
P = 128  # SBUF partitions

_PC_RANGE = np.asarray([-51.2, -51.2, -5.0, 51.2, 51.2, 3.0], dtype=np.float32)
_DIMS = _PC_RANGE[3:] - _PC_RANGE[:3]
_EFF_MIN, _EFF_MAX = np.float32(1.0), np.float32(6.0)

_NC_CACHE = {}
_CFG = {}


def _chunk_plan(n_pad):
    """Gather chunk sizes (multiples of 256; small last chunk for the tail)."""
    plan = []
    rem = n_pad
    while rem > 1024:
        plan.append(768)
        rem -= 768
    if rem > 256:
        plan.append(rem - 256)
        rem = 256
    plan.append(rem)
    return plan


def _round_n_pad(n):
    return max(256, -(-n // 256) * 256)


def _build_bass_program(n_pad):
    """Per-core program: gather n_pad covered rows of atten [32768, 128] f32,
    bn_stats per pair -> s_out [128, (n_pad/256)*6] f32."""
    import concourse.bacc as bacc
    import concourse.mybir as mybir
    from concourse import tile

    f32 = mybir.dt.float32
    i16 = mybir.dt.int16

    chunks = _chunk_plan(n_pad)
    npairs = n_pad // 256  # bn_stats pairs per partition

    nc = bacc.Bacc(
        "TRN2",
        target_bir_lowering=False,
        debug=False,
        dynamic_dma_scratch_size=65536,
    )
    atten = nc.dram_tensor("atten", [HALF, D], f32, kind="ExternalInput")
    gidx = nc.dram_tensor("gidx", [16, n_pad // 16], i16, kind="ExternalInput")
    s_out = nc.dram_tensor("s_out", [P, npairs * 6], f32, kind="ExternalOutput")

    def pair_bn_stats(in_ap, out_ap):
        # raw InstBNStats: whole-scan even/odd stats (6 per partition)
        inst = mybir.InstBNStats(
            name=nc.get_next_instruction_name(),
            ins=[nc.vector.lower_ap(in_ap)],
            outs=[nc.vector.lower_ap(out_ap)],
        )
        return nc.vector.add_instruction(inst)

    with tile.TileContext(nc) as tc:
        with (
            tc.tile_pool(name="io", bufs=len(chunks) + 1) as io_pool,
            tc.tile_pool(name="acc", bufs=1) as acc_pool,
        ):
            idx_t = acc_pool.tile([16, n_pad // 16], i16, tag="idx")
            nc.sync.dma_start(out=idx_t[:], in_=gidx[:, :])
            stats = acc_pool.tile([P, npairs * 6], f32, tag="stats")

            off = 0
            for C in chunks:
                gout = io_pool.tile([P, (C // 128) * D], f32, tag=f"g{C}")
                nc.gpsimd.dma_gather(
                    gout[:].rearrange("p (s d) -> p s d", d=D),
                    atten[:, :],
                    idx_t[:, off // 16 : (off + C) // 16],
                    C,
                    C,
                    D,
                )
                for j in range(C // 256):
                    pr = off // 256 + j
                    in3 = gout[:, (2 * j) * D : (2 * j + 2) * D].rearrange(
                        "p (c d) -> p d c", c=2
                    )
                    pair_bn_stats(in3, stats[:, pr * 6 : (pr + 1) * 6])
                off += C

            nc.sync.dma_start(out=s_out[:, :], in_=stats[:])

    nc.compile()
    return nc


def _get_nc(n_pad=None):
    if n_pad is None:
        n_pad = _NC_CACHE.get("last", 1792)
    if n_pad not in _NC_CACHE:
        _NC_CACHE[n_pad] = _build_bass_program(n_pad)
    _NC_CACHE["last"] = n_pad
    return _NC_CACHE[n_pad]


# ---------------------------------------------------------------------------
# Host-side box logic (exact fp32 replication of the reference semantics)
# ---------------------------------------------------------------------------
def _grid_axis_vals():
    gx = (np.arange(W, dtype=np.float32) + np.float32(0.5)) / np.float32(W) * _DIMS[
        0
    ] + _PC_RANGE[0]
    gy = (np.arange(H, dtype=np.float32) + np.float32(0.5)) / np.float32(H) * _DIMS[
        1
    ] + _PC_RANGE[1]
    return gx, gy


_CORNERS_NORM = np.asarray(
    [[-0.5, -0.5], [-0.5, 0.5], [0.5, 0.5], [0.5, -0.5]], dtype=np.float32
)


def _scene_flags(boxes: np.ndarray, gx: np.ndarray, gy: np.ndarray):
    """Exact per-cell owner flags (reference scan order, argmin tie-break)."""
    centers = boxes[:, :2]
    lw = boxes[:, 3:5]
    angles = boxes[:, 6]
    ratio_l = np.clip(_DIMS[0] / np.float32(W) / lw[:, 0], _EFF_MIN, _EFF_MAX)
    ratio_w = np.clip(_DIMS[1] / np.float32(H) / lw[:, 1], _EFF_MIN, _EFF_MAX)
    eff = np.stack([lw[:, 0] * ratio_l, lw[:, 1] * ratio_w], axis=1)
    corners = eff[:, None, :] * _CORNERS_NORM  # [M, 4, 2]
    c = np.cos(angles)[:, None]
    s = np.sin(angles)[:, None]
    rx = corners[..., 0] * c + corners[..., 1] * s
    ry = -corners[..., 0] * s + corners[..., 1] * c
    corners = np.stack([rx, ry], axis=-1) + centers[:, None, :]  # [M, 4, 2]
    edges = np.roll(corners, -1, axis=1) - corners

    # exact argmin (first-index tie-break) of d2 over the full grid, as in ref
    d2 = (gx[None, None, :] - centers[:, 0:1, None]) ** 2 + (
        gy[None, :, None] - centers[:, 1:2, None]
    ) ** 2  # [M, H, W] f32
    nearest_g = np.argmin(d2.reshape(M, HW), axis=1)

    flag = np.full(HW, -1, dtype=np.int32)
    for i in range(M):
        cmin, cmax = corners[i, :, 0].min(), corners[i, :, 0].max()
        rmin, rmax = corners[i, :, 1].min(), corners[i, :, 1].max()
        c0 = max(0, int(np.searchsorted(gx, cmin)) - 1)
        c1 = min(W, int(np.searchsorted(gx, cmax)) + 1)
        r0 = max(0, int(np.searchsorted(gy, rmin)) - 1)
        r1 = min(H, int(np.searchsorted(gy, rmax)) + 1)
        dx = gx[None, None, c0:c1] - corners[i, :, 0][:, None, None]
        dy = gy[None, r0:r1, None] - corners[i, :, 1][:, None, None]
        cross = (
            edges[i, :, 0][:, None, None] * dy - edges[i, :, 1][:, None, None] * dx
        )
        inside = np.all(cross >= 0, axis=0) | np.all(cross <= 0, axis=0)
        rr, cc = np.nonzero(inside)
        gidx = (rr + r0).astype(np.int64) * W + (cc + c0)
        gidx = np.union1d(gidx, np.asarray([nearest_g[i]]))
        cur = flag[gidx]
        flag[gidx] = np.where(cur == -1, np.int32(i), np.int32(-1))
    return flag


def _reduce_scene(v: np.ndarray, flag: np.ndarray):
    sums = np.zeros(M, dtype=np.float32)
    cnts = np.zeros(M, dtype=np.float32)
    msk = flag >= 0
    np.add.at(sums, flag[msk], v[msk])
    np.add.at(cnts, flag[msk], np.float32(1.0))
    sums *= np.float32(1.0 / 127.0)  # device emits M2; unbiased var = M2/127
    valid = cnts > 0
    box_mean = sums / np.maximum(cnts, np.float32(1.0))
    loss = -np.sum(box_mean[valid], dtype=np.float32)
    return loss, np.float32(np.sum(valid))


def _wrap_idxs(idxs: np.ndarray, n_pad: int):
    arr = np.zeros(n_pad, dtype=np.int16)
    arr[: len(idxs)] = idxs.astype(np.int16)
    return np.ascontiguousarray(arr.reshape(n_pad // 16, 16).T)


def _decode_stats(st: np.ndarray, idxs: np.ndarray, n_pad: int):
    """st [P, npairs, 6] -> v values for each gathered position i:
    i -> partition i%128, slot i//128; pair slot//2, lane slot%2."""
    npos = len(idxs)
    i = np.arange(npos)
    p = i % 128
    slot = i // 128
    vvals = np.where(
        slot % 2 == 0,
        st[p, slot // 2, 2],
        st[p, slot // 2, 5],
    ).astype(np.float32)
    return vvals


def kernel(atten_map: np.ndarray, gt_bboxes: np.ndarray, gt_labels: np.ndarray):
    from concourse.bass_utils import run_bass_kernel_spmd

    gt_bboxes = np.asarray(gt_bboxes, dtype=np.float32)
    gx, gy = _grid_axis_vals()
    flags = [_scene_flags(gt_bboxes[b], gx, gy) for b in range(B)]

    # per-core covered-cell index lists (local row ids within the half)
    core_idxs = []
    for c in range(N_CORES):
        b, h = c // 2, c % 2
        cov = flags[b][h * HALF : (h + 1) * HALF] >= 0
        core_idxs.append(np.nonzero(cov)[0])
    n_pad = _round_n_pad(max(len(ix) for ix in core_idxs))

    nc = _get_nc(n_pad)
    in_maps = []
    for c in range(N_CORES):
        b, h = c // 2, c % 2
        chunk = atten_map[b, h * HALF : (h + 1) * HALF, :]
        chunk = np.ascontiguousarray(np.asarray(chunk), dtype=np.float32)
        in_maps.append(
            {"atten": chunk, "gidx": _wrap_idxs(core_idxs[c], n_pad)}
        )
    res = run_bass_kernel_spmd(nc, in_maps, list(range(N_CORES)))

    losses = np.zeros(B, dtype=np.float32)
    nums = np.zeros(B, dtype=np.float32)
    v = np.zeros((B, HW), dtype=np.float32)
    for c in range(N_CORES):
        b, h = c // 2, c % 2
        st = res.results[c]["s_out"].reshape(P, n_pad // 256, 6)
        idxs = core_idxs[c]
        vv = _decode_stats(st, idxs, n_pad)
        v[b, h * HALF + idxs] = vv
    for b in range(B):
        losses[b], nums[b] = _reduce_scene(v[b], flags[b])
    var_loss = np.sum(losses, dtype=np.float32)
    var_pos_num = np.maximum(np.sum(nums, dtype=np.float32), np.float32(1.0))
    return np.asarray(np.float32(var_loss / var_pos_num))
